# revision 1
# baseline (speedup 1.0000x reference)
"""GCN encoder (2x GCNConv + BatchNorm + PReLU) on 8 Trainium2 NeuronCores.

Full inputs in, full outputs out. Internally:
  - nodes sharded contiguously across 8 cores (12500 real rows + pad -> 12544),
  - v' = dinv * (h @ W) computed locally per core (feat-on-partitions layout),
  - AllGather of the v' table (the "halo exchange" -- random graph means the
    halo is essentially everything),
  - per-edge messages fetched with gpsimd.dma_gather (block-local int16 idxs),
  - segment-sum via gpsimd.dma_scatter_add into an HBM accumulator,
  - conv = dinv * acc (dinv[dst] fold), BN stats via free-dim reduce after a
    PE transpose into [feat, rows] layout, stats AllReduce, fused BN+PReLU
    via two ACT Relu passes + one DVE scalar_tensor_tensor.

norm_e = dinv[src]*dinv[dst] is separable, so no per-edge multiply is needed:
dinv[src] is folded into the gathered table, dinv[dst] into the accumulator
read-out.  BatchNorm makes the conv bias irrelevant (it cancels in x - mean),
so b0/b1 are accepted and ignored.
"""

import math

import numpy as np

import concourse.bass as bass
import concourse.bacc as bacc
import concourse.tile as tile
from concourse import mybir
from concourse import bass_utils
from concourse.masks import make_identity
from concourse.bass_interp import get_hw_module

F32 = mybir.dt.float32
I16 = mybir.dt.int16
EPS = 1e-5
NB = 8  # cores


# ---------------------------------------------------------------- host side


def _wrap16(vals: np.ndarray, cap: int) -> np.ndarray:
    """Pack idx list into the [16, cap//16] layout (j -> [j%16, j//16])."""
    assert vals.shape[0] == cap and cap % 16 == 0
    return np.ascontiguousarray(vals.reshape(cap // 16, 16).T)


def preprocess(x: np.ndarray, edge_index: np.ndarray):
    """Shard nodes, partition edges by (dst core, src block), build idx lists."""
    N = x.shape[0]
    nsh = (N + NB - 1) // NB                     # real rows per shard
    SH = ((nsh + 1 + 127) // 128) * 128          # padded rows (>= nsh+1 zero row)
    zero_row = nsh                               # a guaranteed all-zero table row

    # self-loops are handled analytically on-device (conv += dinv*vloc),
    # so only the real edges go through gather/scatter
    src = edge_index[0]
    dst = edge_index[1]
    deg = np.bincount(dst, minlength=N) + 1  # +1: self-loop
    dinv = (1.0 / np.sqrt(deg.astype(np.float64))).astype(np.float32)

    c_of = dst // nsh                            # owning core of each edge
    b_of = src // nsh                            # source block of each edge
    gloc = (src % nsh).astype(np.int64)
    sloc = (dst % nsh).astype(np.int64)

    counts = np.zeros((NB, NB), np.int64)
    per_cb_g = [[None] * NB for _ in range(NB)]
    per_cb_s = [[None] * NB for _ in range(NB)]
    for c in range(NB):
        mc = c_of == c
        bc = b_of[mc]
        gc = gloc[mc]
        sc = sloc[mc]
        for b in range(NB):
            mb = bc == b
            per_cb_g[c][b] = gc[mb]
            per_cb_s[c][b] = sc[mb]
            counts[c, b] = mb.sum()

    # dma_scatter_add loses updates when duplicate dst rows race within one
    # call, so split each (core, block) edge list into rounds: the k-th edge
    # hitting a given dst goes to round k -> every round is duplicate-free
    # and becomes its own scatter call (calls serialize via the acc WAW dep).
    per_cb_r = [[None] * NB for _ in range(NB)]
    nrounds = np.zeros((NB, NB), np.int64)
    round_sizes = [[None] * NB for _ in range(NB)]
    for c in range(NB):
        for b in range(NB):
            s = per_cb_s[c][b]
            r = np.zeros(len(s), np.int64)
            if len(s):
                order = np.argsort(s, kind="stable")
                ss = s[order]
                occ = np.arange(len(ss)) - np.searchsorted(ss, ss, side="left")
                r[order] = occ
            per_cb_r[c][b] = r
            nrounds[c, b] = int(r.max()) + 1 if len(r) else 0
            round_sizes[c][b] = np.bincount(r) if len(r) else np.zeros(0, np.int64)

    # static per-(block, round) padded sizes = max over cores, 128-multiples
    rounds = []
    for b in range(NB):
        R = int(nrounds[:, b].max())
        sizes = []
        for r in range(R):
            m = max(int(round_sizes[c][b][r]) if r < len(round_sizes[c][b])
                    else 0 for c in range(NB))
            sizes.append(((m + 127) // 128) * 128)
        rounds.append([sz for sz in sizes if sz > 0])
    cap_b = [sum(szs) for szs in rounds]
    tot = sum(cap_b)
    band_off = np.cumsum([0] + cap_b)

    gidx = np.zeros((NB, 128, tot // 16), np.int16)
    sidx = np.zeros((NB, 128, tot // 16), np.int16)
    dinv_cols = np.zeros((NB, 128, SH // 128), np.float32)
    x_sh = np.zeros((NB, SH, x.shape[1]), np.float32)
    for c in range(NB):
        for b in range(NB):
            if cap_b[b] == 0:
                continue
            g = np.full(cap_b[b], zero_row, np.int64)
            # pads must NOT hit row 0: their zero-RMWs race with real row-0
            # updates in the same call. Dump them on the unused zero row.
            s = np.full(cap_b[b], zero_row, np.int64)
            off = 0
            rr = per_cb_r[c][b]
            for r, sz in enumerate(rounds[b]):
                m = rr == r
                n = int(m.sum())
                g[off : off + n] = per_cb_g[c][b][m]
                s[off : off + n] = per_cb_s[c][b][m]
                off += sz
            csl = slice(band_off[b] // 16, band_off[b + 1] // 16)
            gidx[c, :, csl] = np.tile(_wrap16(g.astype(np.int16), cap_b[b]),
                                      (8, 1))
            sidx[c, :, csl] = np.tile(_wrap16(s.astype(np.int16), cap_b[b]),
                                      (8, 1))
        lo = c * nsh
        hi = min(lo + nsh, N)
        d = np.zeros(SH, np.float32)
        d[: hi - lo] = dinv[lo:hi]
        dinv_cols[c] = d.reshape(SH // 128, 128).T
        x_sh[c, : hi - lo] = x[lo:hi]

    return dict(
        N=N, nsh=nsh, SH=SH, rounds=tuple(tuple(r) for r in rounds),
        gidx=gidx, sidx=sidx, dinv_cols=dinv_cols, x_sh=x_sh,
    )


# -------------------------------------------------------------- device side


def build_kernel(N: int, SH: int, rounds, D: int = 128):
    """Build the SPMD Bass program (one program, 8 cores).

    rounds[b] = static (padded, 128-multiple) per-round slot counts for the
    edges sourced from block b; scatter calls are one per round.
    """
    nc = bacc.Bacc("TRN2", target_bir_lowering=False, debug=False,
                   num_devices=NB)
    rg = [list(range(NB))]
    NT = SH // 128               # 128-row tiles per shard
    # 512-col chunks over the SH free dim
    chunks = [(o, min(512, SH - o)) for o in range(0, SH, 512)]
    cap_b = [sum(szs) for szs in rounds]
    tot = sum(cap_b)
    band_off = [0]
    for b in range(NB):
        band_off.append(band_off[-1] + cap_b[b])
    # gather chunks per block: greedy-pack round pieces up to CALL_MAX slots
    # per DMA call (bigger calls overflow SWDGE descriptor capacity on HW);
    # rounds are split freely -- any subset of a duplicate-free round is
    # still duplicate-free
    CALL_MAX = 4096
    gchunks = []
    for b in range(NB):
        pieces = []
        off = 0
        for sz in rounds[b]:
            o = 0
            while o < sz:
                pieces.append((off + o, min(CALL_MAX, sz - o)))
                o += CALL_MAX
            off += sz
        lst, cur = [], []
        for (po, psz) in pieces:
            if cur and (po + psz) - cur[0][0] > CALL_MAX:
                lst.append((cur[0][0], cur))
                cur = []
            cur.append((po, psz))
        if cur:
            lst.append((cur[0][0], cur))
        gchunks.append(lst)
    MT = max((sum(sz for _, sz in rl) for gc in gchunks for _, rl in gc),
             default=128)

    x_in = nc.dram_tensor("x", [SH, D], F32, kind="ExternalInput")
    gidx_in = nc.dram_tensor("gidx", [128, tot // 16], I16, kind="ExternalInput")
    sidx_in = nc.dram_tensor("sidx", [128, tot // 16], I16, kind="ExternalInput")
    dinv_in = nc.dram_tensor("dinv_cols", [128, NT], F32, kind="ExternalInput")
    w_in = [nc.dram_tensor(f"w{l}", [D, D], F32, kind="ExternalInput")
            for l in range(2)]
    gam_in = [nc.dram_tensor(f"gamma{l}", [D, 1], F32, kind="ExternalInput")
              for l in range(2)]
    bet_in = [nc.dram_tensor(f"beta{l}", [D, 1], F32, kind="ExternalInput")
              for l in range(2)]
    a_in = [nc.dram_tensor(f"a{l}", [D, 1], F32, kind="ExternalInput")
            for l in range(2)]
    out_t = nc.dram_tensor("out", [SH, D], F32, kind="ExternalOutput")

    vloc = nc.dram_tensor("vloc", [SH, D], F32)
    vfull = nc.dram_tensor("vfull", [NB * SH, D], F32, addr_space="Shared")
    acc = nc.dram_tensor("acc", [SH, D], F32)
    stats_in = nc.dram_tensor("stats_in", [D, 2], F32)
    stats_out = nc.dram_tensor("stats_out", [D, 2], F32, addr_space="Shared")

    acc_r = acc.ap().rearrange("(t p) f -> t p f", p=128)
    out_r = out_t.ap().rearrange("(t p) f -> t p f", p=128)
    x_r = x_in.ap().rearrange("(t p) f -> t p f", p=128)
    vloc_r = vloc.ap().rearrange("(t p) f -> t p f", p=128)

    with tile.TileContext(nc) as tc:
        with (
            tc.tile_pool(name="pers", bufs=1) as PE_,
            tc.tile_pool(name="act", bufs=1) as PA,
            tc.tile_pool(name="msg", bufs=2) as PM,
            tc.tile_pool(name="work", bufs=3) as PW,
            tc.tile_pool(name="small", bufs=2) as PS,
            tc.tile_pool(name="psA", bufs=2, space="PSUM") as PP,
            tc.tile_pool(name="psT", bufs=4, space="PSUM") as PT,
        ):
            ident = PE_.tile([128, 128], F32, tag="ident")
            make_identity(nc, ident[:])
            gidx_sb = PE_.tile([128, tot // 16], I16, tag="gidx")
            nc.sync.dma_start(gidx_sb[:], gidx_in.ap())
            sidx_sb = PE_.tile([128, tot // 16], I16, tag="sidx")
            nc.sync.dma_start(sidx_sb[:], sidx_in.ap())
            dinv_sb = PE_.tile([128, NT], F32, tag="dinv")
            nc.sync.dma_start(dinv_sb[:], dinv_in.ap())
            w_sb, gam_sb, bet_sb, a_sb = [], [], [], []
            for l in range(2):
                w_sb.append(PE_.tile([128, 128], F32, tag=f"w{l}", name=f"w{l}_sb"))
                nc.sync.dma_start(w_sb[l][:], w_in[l].ap())
                gam_sb.append(PE_.tile([128, 1], F32, tag=f"g{l}", name=f"g{l}_sb"))
                nc.sync.dma_start(gam_sb[l][:], gam_in[l].ap())
                bet_sb.append(PE_.tile([128, 1], F32, tag=f"b{l}", name=f"b{l}_sb"))
                nc.sync.dma_start(bet_sb[l][:], bet_in[l].ap())
                a_sb.append(PE_.tile([128, 1], F32, tag=f"a{l}", name=f"a{l}_sb"))
                nc.sync.dma_start(a_sb[l][:], a_in[l].ap())
            zero_sb = PE_.tile([128, 128], F32, tag="zero")
            nc.vector.memset(zero_sb[:], 0.0)
            eps_sb = PE_.tile([128, 1], F32, tag="eps")
            nc.vector.memset(eps_sb[:], EPS)

            actT = PA.tile([128, SH], F32, tag="actT")  # h_l as [feat, rows]

            # ---- load x, transpose into actT
            for t in range(NT):
                xt = PW.tile([128, 128], F32, tag="xt")
                nc.sync.dma_start(xt[:], x_r[t])
                tp = PT.tile([128, 128], F32, tag="tp")
                nc.tensor.transpose(out=tp[:], in_=xt[:], identity=ident[:])
                nc.vector.tensor_copy(actT[:, 128 * t : 128 * (t + 1)], tp[:])

            for l in range(2):
                # ---- v = W.T-free matmul: vT[fout, rows] = w[fin,fout].T @ actT
                for (o, cw) in chunks:
                    vp = PP.tile([128, 512], F32, tag="vp")
                    nc.tensor.matmul(out=vp[:, :cw], lhsT=w_sb[l][:],
                                     rhs=actT[:, o : o + cw],
                                     start=True, stop=True)
                    vt = PW.tile([128, 512], F32, tag="vt")
                    nc.vector.tensor_copy(vt[:, :cw], vp[:, :cw])
                    # transpose each 128-tile back to [rows, feat], fold dinv[src]
                    for s in range(0, cw, 128):
                        t = (o + s) // 128
                        tp = PT.tile([128, 128], F32, tag="tp")
                        nc.tensor.transpose(out=tp[:], in_=vt[:, s : s + 128],
                                            identity=ident[:])
                        vv = PW.tile([128, 128], F32, tag="vv")
                        nc.vector.tensor_scalar(
                            vv[:], tp[:], dinv_sb[:, t : t + 1], None,
                            op0=mybir.AluOpType.mult)
                        nc.sync.dma_start(vloc_r[t], vv[:])

                # ---- halo exchange: AllGather the v' table
                nc.gpsimd.collective_compute(
                    "AllGather", mybir.AluOpType.bypass, replica_groups=rg,
                    ins=[vloc.ap().opt()], outs=[vfull.ap().opt()])

                # ---- zero accumulator
                for t in range(NT):
                    nc.sync.dma_start(acc_r[t], zero_sb[:])

                # ---- gather messages, scatter-add into acc
                # (one scatter call per duplicate-free round; WAW on acc
                # serializes the RMWs so no same-row races)
                for b in range(NB):
                    for (goff, rlist) in gchunks[b]:
                        gsz = sum(sz for _, sz in rlist)
                        mt = PM.tile([128, MT // 128, 128], F32, tag="mt")
                        mtv = mt[:, : gsz // 128, :]
                        isl = slice((band_off[b] + goff) // 16,
                                    (band_off[b] + goff + gsz) // 16)
                        nc.gpsimd.dma_gather(
                            out_ap=mtv,
                            in_ap=vfull.ap()[b * SH : (b + 1) * SH, :],
                            idxs_ap=gidx_sb[:, isl],
                            num_idxs=gsz, num_idxs_reg=gsz, elem_size=D,
                            single_packet=False)
                        for (roff, rsz) in rlist:
                            ssl = slice((band_off[b] + roff) // 16,
                                        (band_off[b] + roff + rsz) // 16)
                            lo = (roff - goff) // 128
                            nc.gpsimd.dma_scatter_add(
                                out_ap=acc.ap()[:, :],
                                in_ap=mt[:, lo : lo + rsz // 128, :],
                                idxs_ap=sidx_sb[:, ssl],
                                num_idxs=rsz, num_idxs_reg=rsz, elem_size=D,
                                single_packet=False)

                # ---- conv = dinv[dst] * (acc + vloc); transpose into actT
                # (acc + vloc adds the self-loop term dinv[i]^2 * v[i])
                for t in range(NT):
                    at = PW.tile([128, 128], F32, tag="at")
                    nc.sync.dma_start(at[:], acc_r[t])
                    vl = PW.tile([128, 128], F32, tag="vl")
                    nc.sync.dma_start(vl[:], vloc_r[t])
                    sc = PW.tile([128, 128], F32, tag="sc")
                    nc.vector.tensor_tensor(out=sc[:], in0=at[:], in1=vl[:],
                                            op=mybir.AluOpType.add)
                    nc.vector.tensor_scalar(
                        sc[:], sc[:], dinv_sb[:, t : t + 1], None,
                        op0=mybir.AluOpType.mult)
                    tp = PT.tile([128, 128], F32, tag="tp")
                    nc.tensor.transpose(out=tp[:], in_=sc[:], identity=ident[:])
                    nc.vector.tensor_copy(actT[:, 128 * t : 128 * (t + 1)], tp[:])

                # ---- BN stats (biased, over the real N rows; pad rows are 0)
                nk = len(chunks)
                sumc = PS.tile([128, nk], F32, tag="sumc")
                sqc = PS.tile([128, nk], F32, tag="sqc")
                for k, (o, cw) in enumerate(chunks):
                    nc.vector.tensor_reduce(
                        out=sumc[:, k : k + 1], in_=actT[:, o : o + cw],
                        axis=mybir.AxisListType.X, op=mybir.AluOpType.add)
                    sq = PW.tile([128, 512], F32, tag="sq")
                    nc.scalar.activation(
                        out=sq[:, :cw], in_=actT[:, o : o + cw],
                        func=mybir.ActivationFunctionType.Square,
                        bias=zero_sb[:, 0:1],
                        accum_out=sqc[:, k : k + 1])
                stats_sb = PS.tile([128, 2], F32, tag="stats")
                nc.vector.tensor_reduce(out=stats_sb[:, 0:1], in_=sumc[:],
                                        axis=mybir.AxisListType.X,
                                        op=mybir.AluOpType.add)
                nc.vector.tensor_reduce(out=stats_sb[:, 1:2], in_=sqc[:],
                                        axis=mybir.AxisListType.X,
                                        op=mybir.AluOpType.add)
                nc.sync.dma_start(stats_in.ap(), stats_sb[:])
                nc.gpsimd.collective_compute(
                    "AllReduce", mybir.AluOpType.add, replica_groups=rg,
                    ins=[stats_in.ap().opt()], outs=[stats_out.ap().opt()])
                stats2 = PS.tile([128, 2], F32, tag="stats2")
                nc.sync.dma_start(stats2[:], stats_out.ap())

                # ---- BN affine params ([128,1] each)
                mu = PS.tile([128, 1], F32, tag="mu")
                nc.vector.tensor_scalar(mu[:], stats2[:, 0:1], 1.0 / N, None,
                                        op0=mybir.AluOpType.mult)
                e2 = PS.tile([128, 1], F32, tag="e2")
                nc.vector.tensor_scalar(e2[:], stats2[:, 1:2], 1.0 / N, None,
                                        op0=mybir.AluOpType.mult)
                var = PS.tile([128, 1], F32, tag="var")
                nc.vector.scalar_tensor_tensor(
                    out=var[:], in0=mu[:], scalar=-1.0, in1=mu[:],
                    op0=mybir.AluOpType.mult, op1=mybir.AluOpType.mult)
                nc.vector.tensor_tensor(out=var[:], in0=e2[:], in1=var[:],
                                        op=mybir.AluOpType.add)
                sd = PS.tile([128, 1], F32, tag="sd")
                nc.scalar.activation(out=sd[:], in_=var[:],
                                     func=mybir.ActivationFunctionType.Sqrt,
                                     bias=eps_sb[:, 0:1])
                rinv = PS.tile([128, 1], F32, tag="rinv")
                nc.vector.reciprocal(rinv[:], sd[:])
                alpha = PS.tile([128, 1], F32, tag="alpha")
                nc.vector.tensor_tensor(out=alpha[:], in0=gam_sb[l][:],
                                        in1=rinv[:], op=mybir.AluOpType.mult)
                bias_p = PS.tile([128, 1], F32, tag="biasp")
                # bias' = beta - alpha*mu
                nc.vector.scalar_tensor_tensor(
                    out=bias_p[:], in0=alpha[:], scalar=-1.0, in1=mu[:],
                    op0=mybir.AluOpType.mult, op1=mybir.AluOpType.mult)
                nc.vector.tensor_tensor(out=bias_p[:], in0=bet_sb[l][:],
                                        in1=bias_p[:], op=mybir.AluOpType.add)
                nalpha = PS.tile([128, 1], F32, tag="nalpha")
                nc.vector.tensor_scalar(nalpha[:], alpha[:], -1.0, None,
                                        op0=mybir.AluOpType.mult)
                nbias = PS.tile([128, 1], F32, tag="nbias")
                nc.vector.tensor_scalar(nbias[:], bias_p[:], -1.0, None,
                                        op0=mybir.AluOpType.mult)
                na = PS.tile([128, 1], F32, tag="na")
                nc.vector.tensor_scalar(na[:], a_sb[l][:], -1.0, None,
                                        op0=mybir.AluOpType.mult)

                # ---- fused BN + PReLU: y = relu(z) - a*relu(-z), z = alpha*x+bias'
                for (o, cw) in chunks:
                    pos = PW.tile([128, 512], F32, tag="pos")
                    nc.scalar.activation(
                        out=pos[:, :cw], in_=actT[:, o : o + cw],
                        func=mybir.ActivationFunctionType.Relu,
                        bias=bias_p[:, :1], scale=alpha[:, :1])
                    neg = PW.tile([128, 512], F32, tag="neg")
                    nc.scalar.activation(
                        out=neg[:, :cw], in_=actT[:, o : o + cw],
                        func=mybir.ActivationFunctionType.Relu,
                        bias=nbias[:, :1], scale=nalpha[:, :1])
                    # actT = (neg * (-a)) + pos
                    nc.vector.scalar_tensor_tensor(
                        out=actT[:, o : o + cw], in0=neg[:, :cw],
                        scalar=na[:, :1], in1=pos[:, :cw],
                        op0=mybir.AluOpType.mult, op1=mybir.AluOpType.add)

            # ---- write h2 back as [rows, feat]
            for t in range(NT):
                tp = PT.tile([128, 128], F32, tag="tp")
                nc.tensor.transpose(out=tp[:],
                                    in_=actT[:, 128 * t : 128 * (t + 1)],
                                    identity=ident[:])
                ot = PW.tile([128, 128], F32, tag="ot")
                nc.vector.tensor_copy(ot[:], tp[:])
                nc.sync.dma_start(out_r[t], ot[:])

    nc.compile()
    return nc


# ------------------------------------------------------------------- driver

_CACHE: dict = {}


def _get_compiled(key, N, SH, rounds):
    if key not in _CACHE:
        nc = build_kernel(N, SH, rounds)
        nc.m = get_hw_module(nc.m)
        _CACHE[key] = nc
    return _CACHE[key]


def make_in_maps(pre, w0, b0, gamma0, beta0, a0, w1, b1, gamma1, beta1, a1):
    def col(v):
        return np.ascontiguousarray(np.asarray(v, np.float32).reshape(-1, 1))

    def rep(v):
        return np.full((128, 1), np.float32(np.asarray(v).reshape(-1)[0]),
                       np.float32)

    maps = []
    for c in range(NB):
        maps.append({
            "x": pre["x_sh"][c],
            "gidx": pre["gidx"][c],
            "sidx": pre["sidx"][c],
            "dinv_cols": pre["dinv_cols"][c],
            "w0": np.ascontiguousarray(np.asarray(w0, np.float32)),
            "w1": np.ascontiguousarray(np.asarray(w1, np.float32)),
            "gamma0": col(gamma0), "beta0": col(beta0), "a0": rep(a0),
            "gamma1": col(gamma1), "beta1": col(beta1), "a1": rep(a1),
        })
    return maps


def kernel(x, edge_index, w0, b0, gamma0, beta0, a0,
           w1, b1, gamma1, beta1, a1, _trace=False):
    x = np.asarray(x, np.float32)
    edge_index = np.asarray(edge_index, np.int64)
    pre = preprocess(x, edge_index)
    N, nsh, SH = pre["N"], pre["nsh"], pre["SH"]
    key = (N, SH, pre["rounds"])
    nc = _get_compiled(key, N, SH, pre["rounds"])
    in_maps = make_in_maps(pre, w0, b0, gamma0, beta0, a0,
                           w1, b1, gamma1, beta1, a1)
    res = bass_utils.run_bass_kernel_spmd(
        nc, in_maps, core_ids=list(range(NB)), trace=_trace)
    out = np.concatenate([res.results[c]["out"][:nsh] for c in range(NB)],
                         axis=0)[:N]
    if _trace:
        kernel.last_results = res
    return np.ascontiguousarray(out)



# revision 4
# speedup vs baseline: 2.0358x; 2.0358x over previous
"""GCN encoder (2x GCNConv + BatchNorm + PReLU) on 8 Trainium2 NeuronCores.

Full inputs in, full outputs out. Strategy (v2):
  - nodes sharded contiguously across 8 cores (12500 real rows + pad -> 12544)
  - v = h @ W computed locally per core as row tiles (actT tiles as matmul
    lhsT), written to a bf16 vloc table; AllGather -> vfull (the halo)
  - per-edge messages fetched with gpsimd.dma_gather from vfull, edges sorted
    by destination tile; the vfull table is split into 4 "superblocks" of
    25088 rows so block-local int16 indices cover the whole table
  - the segment-sum (scatter-add) is done on the TensorEngine: for each
    128-message chunk, acc^T[feat, dst] += M[msg, feat].T @ S[msg, dst] where
    S[m, d] = norm_e[m] * (dstl_rel[m] == d) is built by one DVE tensor_scalar
    op (is_equal against an iota row, times the per-edge norm).  Duplicate
    dst rows within a chunk are handled natively by the matmul; PSUM
    accumulates one [feat, 128-dst] tile per destination tile.
  - self-loops are a diag(dinv^2) matmul against the local v row tiles
  - BN stats accumulate for free in the PSUM->SBUF drain (ACT accum_out),
    stats AllReduce, fused BN+PReLU via two ACT Relu passes + one DVE op.

norm_e = dinv[src]*dinv[dst] is precomputed per edge on the host and folded
into the selector, so the v table stays raw (pad rows never pollute: pads
carry norm 0).  BatchNorm makes the conv bias irrelevant, so b0/b1 are
accepted and ignored.  Everything bf16 except BN statistics and PSUM.
"""

import numpy as np

import concourse.bass as bass
import concourse.bacc as bacc
import concourse.tile as tile
from concourse import mybir
from concourse import bass_utils
from concourse.masks import make_identity
from concourse.bass_interp import get_hw_module

try:
    import ml_dtypes
    BF16_NP = ml_dtypes.bfloat16
except Exception:  # pragma: no cover
    BF16_NP = np.float32

F32 = mybir.dt.float32
BF16 = mybir.dt.bfloat16
I16 = mybir.dt.int16
EPS = 1e-5
NB = 8          # cores
NSB = 4         # superblocks over the vfull table (int16 index reach)
CALL = 4096     # rows per dma_gather call (SWDGE ring capacity bound)
PAD_DSTL = -10000.0


# ---------------------------------------------------------------- host side


def _wrap16(vals: np.ndarray, cap: int) -> np.ndarray:
    """Pack idx list into the [16, cap//16] layout (j -> [j%16, j//16])."""
    assert vals.shape[0] == cap and cap % 16 == 0
    return np.ascontiguousarray(vals.reshape(cap // 16, 16).T)


def preprocess(x: np.ndarray, edge_index: np.ndarray):
    N = x.shape[0]
    nsh = (N + NB - 1) // NB
    SH = ((nsh + 127) // 128) * 128
    NT = SH // 128
    VF = NB * SH
    assert VF % NSB == 0
    VSB = VF // NSB
    assert VSB <= 32767, "superblock must fit int16"

    src = np.asarray(edge_index[0], np.int64)
    dst = np.asarray(edge_index[1], np.int64)
    deg = np.bincount(dst, minlength=N) + 1  # +1: self-loop
    dinv = (1.0 / np.sqrt(deg.astype(np.float64))).astype(np.float32)

    c_of = dst // nsh
    vrow = (src // nsh) * SH + (src % nsh)     # row in vfull
    s_of = vrow // VSB
    sloc = vrow % VSB
    dloc = dst % nsh
    t_of = dloc // 128
    norm = (dinv[src] * dinv[dst]).astype(np.float32)

    # per-core, per-(superblock, tile) bucket counts
    counts = np.zeros((NB, NSB, NT), np.int64)
    per_core = []
    for c in range(NB):
        m = c_of == c
        key = s_of[m] * NT + t_of[m]
        counts[c] += np.bincount(key, minlength=NSB * NT).reshape(NSB, NT)
        per_core.append((key, sloc[m], dloc[m], norm[m]))

    # static (shared) segment sizes: max over cores
    seg = counts.max(axis=0)                       # [NSB, NT]
    off = np.zeros((NSB, NT + 1), np.int64)
    off[:, 1:] = np.cumsum(seg, axis=1)
    cap = ((off[:, -1] + 127) // 128) * 128        # [NSB]
    nch = cap // 128
    chbase = np.zeros(NSB + 1, np.int64)
    chbase[1:] = np.cumsum(nch)
    totch = int(chbase[-1])
    tot16 = int(cap.sum() // 16)

    # chunk -> tile incidence (static)
    pri = np.zeros((NSB, int(nch.max())), np.int64)
    incid = [[] for _ in range(NSB)]               # incid[s][j] = [tiles]
    for s in range(NSB):
        for j in range(int(nch[s])):
            lo, hi = 128 * j, 128 * j + 128
            ts = [t for t in range(NT)
                  if seg[s, t] > 0 and off[s, t] < hi and off[s, t + 1] > lo]
            incid[s].append(ts)
            pri[s, j] = ts[0] if ts else NT - 1
            assert all(t - pri[s, j] <= 1 for t in ts), (
                "chunk spans >2 tiles; need general selector path")

    # gather call slices per stream
    calls = []                                     # calls[s] = [(off, sz)]
    for s in range(NSB):
        lst, o = [], 0
        while o < cap[s]:
            lst.append((o, int(min(CALL, cap[s] - o))))
            o += CALL
        calls.append(lst)

    # per-core stream tensors
    gidx = np.zeros((NB, 128, tot16), np.int16)
    dstl = np.full((NB, 128, totch), PAD_DSTL, np.float32)
    normt = np.zeros((NB, 128, totch), np.float32)
    dinv2 = np.zeros((NB, 128, NT), np.float32)
    x_sh = np.zeros((NB, SH, x.shape[1]), np.float32)
    for c in range(NB):
        key, sl, dl, nm = per_core[c]
        order = np.argsort(key, kind="stable")
        key, sl, dl, nm = key[order], sl[order], dl[order], nm[order]
        # position of each edge inside the padded stream
        kcnt = np.bincount(key, minlength=NSB * NT).reshape(NSB, NT)
        # rank within bucket
        rank = np.arange(len(key)) - np.repeat(
            np.concatenate([[0], np.cumsum(np.bincount(key, minlength=NSB * NT))[:-1]]),
            np.bincount(key, minlength=NSB * NT))
        s_e = key // NT
        t_e = key % NT
        pos = off[s_e, t_e] + rank                 # position within stream s_e
        col16 = np.zeros(tot16, np.int16)
        base16 = np.concatenate([[0], np.cumsum(cap // 16)])[:-1]
        for s in range(NSB):
            ms = s_e == s
            g = np.zeros(int(cap[s]), np.int16)
            g[pos[ms]] = sl[ms].astype(np.int16)
            gidx[c, :, int(base16[s]):int(base16[s] + cap[s] // 16)] = np.tile(
                _wrap16(g, int(cap[s])), (8, 1))
            dcol = np.full(int(cap[s]), PAD_DSTL, np.float32)
            prit = pri[s, (pos[ms] // 128)]
            dcol[pos[ms]] = dl[ms] - 128.0 * prit
            ncol = np.zeros(int(cap[s]), np.float32)
            ncol[pos[ms]] = nm[ms]
            csl = slice(int(chbase[s]), int(chbase[s + 1]))
            dstl[c, :, csl] = dcol.reshape(-1, 128).T
            normt[c, :, csl] = ncol.reshape(-1, 128).T
        lo = c * nsh
        hi = min(lo + nsh, N)
        d2 = np.zeros(SH, np.float32)
        d2[: hi - lo] = dinv[lo:hi] ** 2
        dinv2[c] = d2.reshape(NT, 128).T
        x_sh[c, : hi - lo] = x[lo:hi]

    # schedule events (shared across cores)
    contrib = np.ones(NT, np.int64)                # self-loop
    for s in range(NSB):
        for j in range(int(nch[s])):
            contrib[incid[s][j]] += 1
    events = []
    started = np.zeros(NT, bool)
    emitted = set()
    rem = contrib.copy()

    def start_tile(t):
        started[t] = True
        events.append(("tstart", t))
        rem[t] -= 1
        if rem[t] == 0:
            events.append(("tdone", t))

    for t in range(NT):
        if not started[t]:
            start_tile(t)
        if t + 1 < NT and not started[t + 1]:
            start_tile(t + 1)
        for s in range(NSB):
            for j in range(int(nch[s])):
                if pri[s, j] != t or not incid[s][j]:
                    continue
                k = (128 * j) // CALL
                if (s, k) not in emitted:
                    emitted.add((s, k))
                    events.append(("gather", s, k))
                events.append(("sel", s, j))
                for dt in incid[s][j]:
                    rem[dt] -= 1
                    events.append(("mm", s, j, dt, dt - t, rem[dt] == 0))
                    if rem[dt] == 0:
                        events.append(("tdone", dt))

    iota = np.tile(np.arange(256, dtype=np.float32), (128, 1))

    sched = dict(N=N, nsh=nsh, SH=SH, NT=NT, VSB=VSB,
                 cap=tuple(int(v) for v in cap),
                 chbase=tuple(int(v) for v in chbase),
                 calls=tuple(tuple(cl) for cl in calls),
                 events=tuple(events))
    data = dict(gidx=gidx, dstl=dstl, normt=normt, dinv2=dinv2,
                x_sh=x_sh, iota=iota.astype(BF16_NP))
    return sched, data


# -------------------------------------------------------------- device side


def build_kernel(sched):
    N, SH, NT, VSB = sched["N"], sched["SH"], sched["NT"], sched["VSB"]
    cap, chbase, calls, events = (sched["cap"], sched["chbase"],
                                  sched["calls"], sched["events"])
    tot16 = sum(cap) // 16
    totch = sum(cap) // 128
    base16 = [0]
    for s in range(NSB):
        base16.append(base16[-1] + cap[s] // 16)
    chunks512 = [(o, min(512, SH - o)) for o in range(0, SH, 512)]

    nc = bacc.Bacc("TRN2", target_bir_lowering=False, debug=False,
                   num_devices=NB)
    rg = [list(range(NB))]

    x_in = nc.dram_tensor("x", [SH, 128], F32, kind="ExternalInput")
    gidx_in = nc.dram_tensor("gidx", [128, tot16], I16, kind="ExternalInput")
    dstl_in = nc.dram_tensor("dstl", [128, totch], F32, kind="ExternalInput")
    norm_in = nc.dram_tensor("normt", [128, totch], F32, kind="ExternalInput")
    dinv2_in = nc.dram_tensor("dinv2", [128, NT], F32, kind="ExternalInput")
    iota_in = nc.dram_tensor("iota", [128, 256], BF16, kind="ExternalInput")
    w_in = [nc.dram_tensor(f"w{l}", [128, 128], BF16, kind="ExternalInput")
            for l in range(2)]
    gam_in = [nc.dram_tensor(f"gamma{l}", [128, 1], F32, kind="ExternalInput")
              for l in range(2)]
    bet_in = [nc.dram_tensor(f"beta{l}", [128, 1], F32, kind="ExternalInput")
              for l in range(2)]
    a_in = [nc.dram_tensor(f"a{l}", [128, 1], F32, kind="ExternalInput")
            for l in range(2)]
    out_t = nc.dram_tensor("out", [SH, 128], F32, kind="ExternalOutput")

    vloc = nc.dram_tensor("vloc", [SH, 128], BF16)
    vfull = nc.dram_tensor("vfull", [NB * SH, 128], BF16, addr_space="Shared")
    stats_in = nc.dram_tensor("stats_in", [128, 2], F32)
    stats_out = nc.dram_tensor("stats_out", [128, 2], F32, addr_space="Shared")

    x_r = x_in.ap().rearrange("(t p) f -> t p f", p=128)
    vloc_r = vloc.ap().rearrange("(t p) f -> t p f", p=128)
    out_r = out_t.ap().rearrange("(t p) f -> t p f", p=128)

    with tile.TileContext(nc) as tc:
        with (
            tc.tile_pool(name="pers", bufs=1) as PE_,
            tc.tile_pool(name="msg", bufs=2) as PM,
            tc.tile_pool(name="sel", bufs=6) as PSL,
            tc.tile_pool(name="work", bufs=3) as PW,
            tc.tile_pool(name="small", bufs=2) as PS,
            tc.tile_pool(name="psV", bufs=2, space="PSUM") as PV,
            tc.tile_pool(name="psC", bufs=4, space="PSUM") as PC,
            tc.tile_pool(name="psT", bufs=2, space="PSUM") as PT,
        ):
            ident = PE_.tile([128, 128], F32, tag="ident")
            make_identity(nc, ident[:])
            ident_bf = PE_.tile([128, 128], BF16, tag="identbf")
            make_identity(nc, ident_bf[:])
            gidx_sb = PE_.tile([128, tot16], I16, tag="gidx")
            nc.sync.dma_start(gidx_sb[:], gidx_in.ap())
            dstl_sb = PE_.tile([128, totch], F32, tag="dstl")
            nc.sync.dma_start(dstl_sb[:], dstl_in.ap())
            norm_sb = PE_.tile([128, totch], F32, tag="normt")
            nc.sync.dma_start(norm_sb[:], norm_in.ap())
            dinv2_sb = PE_.tile([128, NT], F32, tag="dinv2")
            nc.sync.dma_start(dinv2_sb[:], dinv2_in.ap())
            iota_sb = PE_.tile([128, 256], BF16, tag="iota")
            nc.sync.dma_start(iota_sb[:], iota_in.ap())
            w_sb, gam_sb, bet_sb, a_sb = [], [], [], []
            for l in range(2):
                w_sb.append(PE_.tile([128, 128], BF16, tag=f"w{l}",
                                     name=f"w{l}_sb"))
                nc.sync.dma_start(w_sb[l][:], w_in[l].ap())
                gam_sb.append(PE_.tile([128, 1], F32, tag=f"g{l}",
                                       name=f"g{l}_sb"))
                nc.sync.dma_start(gam_sb[l][:], gam_in[l].ap())
                bet_sb.append(PE_.tile([128, 1], F32, tag=f"b{l}",
                                       name=f"b{l}_sb"))
                nc.sync.dma_start(bet_sb[l][:], bet_in[l].ap())
                a_sb.append(PE_.tile([128, 1], F32, tag=f"a{l}",
                                     name=f"a{l}_sb"))
                nc.sync.dma_start(a_sb[l][:], a_in[l].ap())
            zero_sb = PE_.tile([128, 1], F32, tag="zero")
            nc.vector.memset(zero_sb[:], 0.0)
            eps_sb = PE_.tile([128, 1], F32, tag="eps")
            nc.vector.memset(eps_sb[:], EPS)

            actT = PE_.tile([128, SH], BF16, tag="actT")  # h as [feat, rows]
            vv = [PE_.tile([128, 128], BF16, tag=f"vv{t}", name=f"vv{t}")
                  for t in range(NT)]
            sumc = PE_.tile([128, NT], F32, tag="sumc")
            sqc = PE_.tile([128, NT], F32, tag="sqc")

            # ---- load x, transpose into actT (bf16)
            for t in range(NT):
                xt = PW.tile([128, 128], F32, tag="xt")
                nc.sync.dma_start(xt[:], x_r[t])
                tp = PT.tile([128, 128], F32, tag="tp")
                nc.tensor.transpose(out=tp[:], in_=xt[:], identity=ident[:])
                nc.scalar.activation(
                    out=actT[:, 128 * t: 128 * (t + 1)], in_=tp[:],
                    func=mybir.ActivationFunctionType.Copy)

            for l in range(2):
                # ---- v row tiles: vv[t] = actT[:, t].T @ W  (raw v = h @ W)
                for t in range(NT):
                    vp = PV.tile([128, 128], F32, tag="vp")
                    nc.tensor.matmul(out=vp[:],
                                     lhsT=actT[:, 128 * t: 128 * (t + 1)],
                                     rhs=w_sb[l][:], start=True, stop=True)
                    nc.scalar.activation(
                        out=vv[t][:], in_=vp[:],
                        func=mybir.ActivationFunctionType.Copy)
                    nc.sync.dma_start(vloc_r[t], vv[t][:])

                # ---- halo exchange
                nc.gpsimd.collective_compute(
                    "AllGather", mybir.AluOpType.bypass, replica_groups=rg,
                    ins=[vloc.ap().opt()], outs=[vfull.ap().opt()])

                # ---- consumption: gather + selector matmuls per event list
                psum_t = {}
                mt_bufs = {}
                for ev in events:
                    if ev[0] == "tstart":
                        t = ev[1]
                        pc = PC.tile([128, 128], F32, tag="conv")
                        psum_t[t] = pc
                        diag = PSL.tile([128, 128], BF16, tag="diag")
                        nc.vector.tensor_scalar(
                            diag[:], ident_bf[:], dinv2_sb[:, t: t + 1], None,
                            op0=mybir.AluOpType.mult)
                        nc.tensor.matmul(out=pc[:], lhsT=vv[t][:],
                                         rhs=diag[:], start=True, stop=False)
                    elif ev[0] == "gather":
                        _, s, k = ev
                        coff, csz = calls[s][k]
                        mt = PM.tile([128, CALL // 128, 128], BF16,
                                     tag=f"mt{s}")
                        mt_bufs[(s, k)] = mt
                        isl = slice((base16[s] * 16 + coff) // 16,
                                    (base16[s] * 16 + coff + csz) // 16)
                        nc.gpsimd.dma_gather(
                            out_ap=mt[:, : csz // 128, :],
                            in_ap=vfull.ap()[s * VSB: (s + 1) * VSB, :],
                            idxs_ap=gidx_sb[:, isl],
                            num_idxs=csz, num_idxs_reg=csz, elem_size=128,
                            single_packet=False)
                    elif ev[0] == "sel":
                        _, s, j = ev
                        sl = PSL.tile([128, 256], BF16, tag="sel")
                        psum_t[("sel", s, j)] = sl
                        col = chbase[s] + j
                        nc.vector.tensor_scalar(
                            sl[:], iota_sb[:],
                            dstl_sb[:, col: col + 1],
                            norm_sb[:, col: col + 1],
                            op0=mybir.AluOpType.is_equal,
                            op1=mybir.AluOpType.mult)
                    elif ev[0] == "mm":
                        _, s, j, dt, half, last = ev
                        k = (128 * j) // CALL
                        mt = mt_bufs[(s, k)]
                        ch = j - (k * CALL) // 128
                        sl = psum_t[("sel", s, j)]
                        nc.tensor.matmul(
                            out=psum_t[dt][:],
                            lhsT=mt[:, ch, :],
                            rhs=sl[:, 128 * half: 128 * half + 128],
                            start=False, stop=last)
                    elif ev[0] == "tdone":
                        t = ev[1]
                        pc = psum_t.pop(t)
                        nc.scalar.activation(
                            out=actT[:, 128 * t: 128 * (t + 1)], in_=pc[:],
                            func=mybir.ActivationFunctionType.Copy,
                            accum_out=sumc[:, t: t + 1])
                        sq = PW.tile([128, 128], BF16, tag="sq")
                        nc.scalar.activation(
                            out=sq[:], in_=pc[:],
                            func=mybir.ActivationFunctionType.Square,
                            accum_out=sqc[:, t: t + 1])

                # ---- BN stats -> AllReduce -> affine params
                stats_sb = PS.tile([128, 2], F32, tag="stats")
                nc.vector.tensor_reduce(out=stats_sb[:, 0:1], in_=sumc[:],
                                        axis=mybir.AxisListType.X,
                                        op=mybir.AluOpType.add)
                nc.vector.tensor_reduce(out=stats_sb[:, 1:2], in_=sqc[:],
                                        axis=mybir.AxisListType.X,
                                        op=mybir.AluOpType.add)
                nc.sync.dma_start(stats_in.ap(), stats_sb[:])
                nc.gpsimd.collective_compute(
                    "AllReduce", mybir.AluOpType.add, replica_groups=rg,
                    ins=[stats_in.ap().opt()], outs=[stats_out.ap().opt()])
                stats2 = PS.tile([128, 2], F32, tag="stats2")
                nc.sync.dma_start(stats2[:], stats_out.ap())

                mu = PS.tile([128, 1], F32, tag="mu")
                nc.vector.tensor_scalar(mu[:], stats2[:, 0:1], 1.0 / N, None,
                                        op0=mybir.AluOpType.mult)
                e2 = PS.tile([128, 1], F32, tag="e2")
                nc.vector.tensor_scalar(e2[:], stats2[:, 1:2], 1.0 / N, None,
                                        op0=mybir.AluOpType.mult)
                var = PS.tile([128, 1], F32, tag="var")
                nc.vector.scalar_tensor_tensor(
                    out=var[:], in0=mu[:], scalar=-1.0, in1=mu[:],
                    op0=mybir.AluOpType.mult, op1=mybir.AluOpType.mult)
                nc.vector.tensor_tensor(out=var[:], in0=e2[:], in1=var[:],
                                        op=mybir.AluOpType.add)
                sd = PS.tile([128, 1], F32, tag="sd")
                nc.scalar.activation(out=sd[:], in_=var[:],
                                     func=mybir.ActivationFunctionType.Sqrt,
                                     bias=eps_sb[:, 0:1])
                rinv = PS.tile([128, 1], F32, tag="rinv")
                nc.vector.reciprocal(rinv[:], sd[:])
                alpha = PS.tile([128, 1], F32, tag="alpha")
                nc.vector.tensor_tensor(out=alpha[:], in0=gam_sb[l][:],
                                        in1=rinv[:], op=mybir.AluOpType.mult)
                bias_p = PS.tile([128, 1], F32, tag="biasp")
                nc.vector.scalar_tensor_tensor(
                    out=bias_p[:], in0=alpha[:], scalar=-1.0, in1=mu[:],
                    op0=mybir.AluOpType.mult, op1=mybir.AluOpType.mult)
                nc.vector.tensor_tensor(out=bias_p[:], in0=bet_sb[l][:],
                                        in1=bias_p[:], op=mybir.AluOpType.add)
                nalpha = PS.tile([128, 1], F32, tag="nalpha")
                nc.vector.tensor_scalar(nalpha[:], alpha[:], -1.0, None,
                                        op0=mybir.AluOpType.mult)
                nbias = PS.tile([128, 1], F32, tag="nbias")
                nc.vector.tensor_scalar(nbias[:], bias_p[:], -1.0, None,
                                        op0=mybir.AluOpType.mult)
                na = PS.tile([128, 1], F32, tag="na")
                nc.vector.tensor_scalar(na[:], a_sb[l][:], -1.0, None,
                                        op0=mybir.AluOpType.mult)

                # ---- fused BN + PReLU: y = relu(z) - a*relu(-z)
                for (o, cw) in chunks512:
                    pos = PW.tile([128, 512], BF16, tag="pos")
                    nc.scalar.activation(
                        out=pos[:, :cw], in_=actT[:, o: o + cw],
                        func=mybir.ActivationFunctionType.Relu,
                        bias=bias_p[:, :1], scale=alpha[:, :1])
                    neg = PW.tile([128, 512], BF16, tag="neg")
                    nc.scalar.activation(
                        out=neg[:, :cw], in_=actT[:, o: o + cw],
                        func=mybir.ActivationFunctionType.Relu,
                        bias=nbias[:, :1], scale=nalpha[:, :1])
                    nc.vector.scalar_tensor_tensor(
                        out=actT[:, o: o + cw], in0=neg[:, :cw],
                        scalar=na[:, :1], in1=pos[:, :cw],
                        op0=mybir.AluOpType.mult, op1=mybir.AluOpType.add)

            # ---- write h2 back as [rows, feat] f32
            for t in range(NT):
                tp = PT.tile([128, 128], F32, tag="tp")
                nc.tensor.matmul(out=tp[:],
                                 lhsT=actT[:, 128 * t: 128 * (t + 1)],
                                 rhs=ident_bf[:], start=True, stop=True)
                ot = PW.tile([128, 128], F32, tag="ot")
                nc.scalar.activation(
                    out=ot[:], in_=tp[:],
                    func=mybir.ActivationFunctionType.Copy)
                nc.sync.dma_start(out_r[t], ot[:])

    nc.compile()
    return nc


# ------------------------------------------------------------------- driver

_CACHE: dict = {}


def _get_compiled(sched):
    key = (sched["N"], sched["SH"], sched["cap"], sched["events"])
    if key not in _CACHE:
        nc = build_kernel(sched)
        nc.m = get_hw_module(nc.m)
        _CACHE[key] = nc
    return _CACHE[key]


def make_in_maps(sched, data, w0, gamma0, beta0, a0, w1, gamma1, beta1, a1):
    def col(v):
        return np.ascontiguousarray(np.asarray(v, np.float32).reshape(-1, 1))

    def rep(v):
        return np.full((128, 1), np.float32(np.asarray(v).reshape(-1)[0]),
                       np.float32)

    w0b = np.ascontiguousarray(np.asarray(w0, np.float32)).astype(BF16_NP)
    w1b = np.ascontiguousarray(np.asarray(w1, np.float32)).astype(BF16_NP)
    maps = []
    for c in range(NB):
        maps.append({
            "x": data["x_sh"][c],
            "gidx": data["gidx"][c],
            "dstl": data["dstl"][c],
            "normt": data["normt"][c],
            "dinv2": data["dinv2"][c],
            "iota": data["iota"],
            "w0": w0b, "w1": w1b,
            "gamma0": col(gamma0), "beta0": col(beta0), "a0": rep(a0),
            "gamma1": col(gamma1), "beta1": col(beta1), "a1": rep(a1),
        })
    return maps


def kernel(x, edge_index, w0, b0, gamma0, beta0, a0,
           w1, b1, gamma1, beta1, a1, _trace=False):
    x = np.asarray(x, np.float32)
    edge_index = np.asarray(edge_index, np.int64)
    sched, data = preprocess(x, edge_index)
    nc = _get_compiled(sched)
    in_maps = make_in_maps(sched, data, w0, gamma0, beta0, a0,
                           w1, gamma1, beta1, a1)
    res = bass_utils.run_bass_kernel_spmd(
        nc, in_maps, core_ids=list(range(NB)), trace=_trace)
    nsh, N = sched["nsh"], sched["N"]
    out = np.concatenate([res.results[c]["out"][:nsh] for c in range(NB)],
                         axis=0)[:N]
    if _trace:
        kernel.last_results = res
    return np.ascontiguousarray(out)
